# revision 1
# baseline (speedup 1.0000x reference)
"""Trainium2 Bass kernel for ClassLinearWithLORA — fp8 DoubleRow version.

out = x @ W.T + b + gates[-1] * (alpha * (x @ A[-1]) @ B_lora[-1])

Strategy (per core, data-parallel over 8 cores / 1024 rows each):
  - K=1024 contraction split into G=4 groups of 256 rows, run as fp8(e4m3)
    DoubleRow matmuls: one 256-cycle pass contracts 256 K-rows (vs fp32r's
    128 K-rows per 512 cycles — 4x the contraction rate).
  - Per-group compensated-precision LEVELS[g]:
      1-term:  xh.wh                 (rel-err contrib ~1.5e-2 per group)
      2-term:  + xh.wl               (~1.1e-2)
      3-term:  + xl.wh               (~1e-3)
    with vh = e4m3(v), vl = e4m3(v - vh) (unscaled residual; its subnormal
    quantization is second-order). W/A/B/b carry a global SW=32 pre-scale
    (epilogue multiplies by 1/SW) to center W-side values in e4m3 range.
  - u' = SW*g*(x @ A[-1]) via 3-term fp8 DR with the A operand's 16 columns
    TRIPLICATED (ps_l is [48, 512]: three device-writable copies of u on
    partitions 0-47). The vector gate-multiply writes uh=e4m3(g*u) twice and
    ul=e4m3(g*u - uh) once into the aug lhsT.
  - The whole LoRA + bias epilogue is ONE fp8 DR pass per psum tile
    (49 partitions x 2 = 98 K-rows: uh.Bh, uh.Bl, ul.Bh, ones.bias_hi/lo),
    256 cycles instead of a 512-cycle fp32r K=17 pass.
  - Epilogue: tensor_scalar_mul by 1/SW psum->sbuf fp16, DMA out. Final
    output block alternates DVE/ACT engines and both DMA rings to cut the
    drain tail (ACT activation table preloaded in the prologue).
  - Pacing: x hi/lo layers stream per-group on the ACT ring (first chunk
    split in halves); W block 0 per group on the SP ring; per group the PE
    runs hi passes (xh/wh only) before lo passes. ob=1 weights ride the ACT
    ring behind x so they don't steal prologue bandwidth from it.

LEVELS=(1,3,3,3): measured end-to-end rel-err 1.52e-2 (deterministic seeded
inputs; harness gate 2e-2). PE sum = 64*11*256 + 24*256 cyc = 75.9us.
"""

import numpy as np
import ml_dtypes

import concourse.bacc as bacc
import concourse.mybir as mybir
import concourse.tile as tile
from concourse.bass_utils import run_bass_kernel_spmd

F32 = mybir.dt.float32
F32R = mybir.dt.float32r
F16 = mybir.dt.float16
F8 = mybir.dt.float8e4
FP8NP = ml_dtypes.float8_e4m3
DR = mybir.MatmulPerfMode.DoubleRow

N_CORES = 8
B, S, D_IN, D_OUT, R_LORA = 4, 2048, 1024, 4096, 16
ROWS = B * S                  # 8192
R_CORE = ROWS // N_CORES      # 1024 rows per core
G = 4                         # K groups of 256
NB = 512                      # moving free dim
OB = D_OUT // NB              # 8 output blocks
RT = R_CORE // 128            # 8 row tiles per core
UL = 3 * R_LORA               # triplicated u partitions (48)
KA8 = UL + 1                  # aug DR partitions (49)
SW = 32.0                     # W-side global scale
E4M3_MAX = 240.0

LEVELS = (1, 3, 3, 3)
G2 = tuple(g for g in range(G) if LEVELS[g] >= 2)   # groups with a wl layer
NG2 = len(G2)


def _to_fp8(a):
    return np.clip(np.asarray(a, np.float32), -E4M3_MAX, E4M3_MAX).astype(FP8NP)


def _split2(v):
    vh = _to_fp8(v)
    vl = _to_fp8(np.asarray(v, np.float32) - vh.astype(np.float32))
    return vh, vl


def _pack_k(a, tail_shape):
    """[1024K, *tail] -> [128, G, 2, *tail]"""
    return np.ascontiguousarray(
        a.reshape(G, 2, 128, *tail_shape).transpose(2, 0, 1, *range(3, 3 + len(tail_shape)))
    )


def _build(wt_bufs: int = 3, psum_bufs: int = 8, out_bufs: int = 4):
    nc = bacc.Bacc(None, target_bir_lowering=False)

    xh_d = nc.dram_tensor("xh", [128, G, 2, R_CORE], F8, kind="ExternalInput")
    xl_d = nc.dram_tensor("xl", [128, G, 2, R_CORE], F8, kind="ExternalInput")
    wh_d = nc.dram_tensor("wh", [128, OB, G, 2, NB], F8, kind="ExternalInput")
    wl_d = nc.dram_tensor("wl", [128, OB, NG2, 2, NB], F8, kind="ExternalInput")
    ah_d = nc.dram_tensor("ah", [128, G, 2, UL], F8, kind="ExternalInput")
    al_d = nc.dram_tensor("al", [128, G, 2, UL], F8, kind="ExternalInput")
    rhs_d = nc.dram_tensor("aug_rhs8", [KA8, 2, D_OUT], F8, kind="ExternalInput")
    one_d = nc.dram_tensor("aug_ones", [1, 2, R_CORE], F8, kind="ExternalInput")
    g_d = nc.dram_tensor("g_rep", [UL, R_CORE], F32, kind="ExternalInput")
    out_d = nc.dram_tensor("out", [R_CORE, D_OUT], F16, kind="ExternalOutput")

    with tile.TileContext(nc) as tc:
        with (
            tc.tile_pool(name="resident", bufs=1) as res,
            tc.tile_pool(name="wpool", bufs=wt_bufs) as wpool,
            tc.tile_pool(name="opool", bufs=out_bufs) as opool,
            tc.tile_pool(name="psum", bufs=psum_bufs, space="PSUM") as psum,
        ):
            # ---- resident loads ------------------------------------------------
            # SP ring: A layers, then per-group W chunks for ob=0 (hi before
            # lo); smalls consumed late (g/rhs/template) go last.
            ah_sb = res.tile([128, G, 2, UL], F8)
            al_sb = res.tile([128, G, 2, UL], F8)
            nc.sync.dma_start(ah_sb[:], ah_d.ap())
            nc.sync.dma_start(al_sb[:], al_d.ap())
            wh0 = wpool.tile([128, G, 2, NB], F8, tag="wh")
            wl0 = wpool.tile([128, NG2, 2, NB], F8, tag="wl")
            nc.sync.dma_start(wh0[:, 0], wh_d.ap()[:, 0, 0])
            # smalls next: the gate chain (and first augs) depend on them
            g_sb = res.tile([UL, R_CORE], F32)
            nc.sync.dma_start(g_sb[:], g_d.ap())
            aug8 = res.tile([KA8, 2, R_CORE], F8)
            # engine ops need partition base 0/32/64/96: zero rows 0-47 in one
            # memset (rows 16-31 and lane-1 of 32-47 stay zero), ones via DMA
            nc.vector.memset(aug8[0:UL, :, :], 0.0)
            nc.sync.dma_start(aug8[UL : UL + 1], one_d.ap())
            rhs_sb = res.tile([KA8, 2, D_OUT], F8)
            nc.sync.dma_start(rhs_sb[:], rhs_d.ap())
            for g in range(1, G):
                nc.sync.dma_start(wh0[:, g], wh_d.ap()[:, 0, g])
                if g in G2:
                    gi = G2.index(g)
                    nc.sync.dma_start(wl0[:, gi], wl_d.ap()[:, 0, gi])
            if 0 in G2:
                nc.sync.dma_start(wl0[:, G2.index(0)], wl_d.ap()[:, 0, G2.index(0)])
            # scratch for the ul computation (rows 32:48 used)
            uh3_sb = res.tile([UL, R_CORE], F8)
            ug3_sb = res.tile([UL, R_CORE], F32)

            # ACT ring: x layers per group, hi before lo; first chunk halved
            xh_sb = res.tile([128, G, 2, R_CORE], F8)
            xl_sb = res.tile([128, G, 2, R_CORE], F8)
            hr = R_CORE // 2
            nc.scalar.dma_start(xh_sb[:, 0, :, 0:hr], xh_d.ap()[:, 0, :, 0:hr])
            nc.scalar.dma_start(xh_sb[:, 0, :, hr:R_CORE], xh_d.ap()[:, 0, :, hr:R_CORE])
            nc.scalar.dma_start(xl_sb[:, 0], xl_d.ap()[:, 0])
            for g in range(1, G):
                nc.scalar.dma_start(xh_sb[:, g], xh_d.ap()[:, g])
                nc.scalar.dma_start(xl_sb[:, g], xl_d.ap()[:, g])

            # preload the ACT activation table off the critical path
            act_warm = res.tile([1, 16], F32)
            nc.scalar.activation(
                act_warm[:], g_sb[0:1, 0:16], mybir.ActivationFunctionType.Copy,
                scale=1.0,
            )

            # ob=1 weights ride the ACT ring directly behind x — issued here
            # (before any stores are emitted) so the FIFO ring starts them as
            # soon as the x transfers finish, without stealing SP prologue
            # bandwidth.
            wh1 = wpool.tile([128, G, 2, NB], F8, tag="wh", name="wh1")
            wl1 = wpool.tile([128, NG2, 2, NB], F8, tag="wl", name="wl1")
            nc.scalar.dma_start(wh1[:], wh_d.ap()[:, 1])
            nc.scalar.dma_start(wl1[:], wl_d.ap()[:, 1])

            def emit_main_hi(ps, rt, g, wh, start):
                rs = slice(rt * 128, (rt + 1) * 128)
                nc.tensor.matmul(
                    ps[:], xh_sb[:, g, :, rs], wh[:, g], perf_mode=DR,
                    start=start, stop=False,
                )

            def emit_main_lo(ps, rt, g, wh, wl):
                rs = slice(rt * 128, (rt + 1) * 128)
                if LEVELS[g] >= 2:
                    nc.tensor.matmul(
                        ps[:], xh_sb[:, g, :, rs], wl[:, G2.index(g)], perf_mode=DR,
                        start=False, stop=False,
                    )
                if LEVELS[g] >= 3:
                    nc.tensor.matmul(
                        ps[:], xl_sb[:, g, :, rs], wh[:, g], perf_mode=DR,
                        start=False, stop=False,
                    )

            def emit_aug(ps, rt, ob):
                # one fp8 DR pass: uh.Bh + uh.Bl + ul.Bh + ones.(bh|bl)
                nc.tensor.matmul(
                    ps[:],
                    aug8[:, :, rt * 128 : (rt + 1) * 128],
                    rhs_sb[:, :, ob * NB : (ob + 1) * NB],
                    perf_mode=DR,
                    start=False,
                    stop=True,
                )

            def emit_epilogue(ps, rt, ob):
                o_sb = opool.tile([128, NB], F16, tag="o_sb", name=f"o_{ob}_{rt}")
                orow = out_d.ap()[rt * 128 : (rt + 1) * 128, ob * NB : (ob + 1) * NB]
                if ob == OB - 1:
                    # tail: alternate compute engine and DMA ring per row tile
                    if rt % 2 == 0:
                        nc.vector.tensor_scalar_mul(o_sb[:], ps[:], 1.0 / SW)
                        nc.scalar.dma_start(orow[:], o_sb[:])
                    else:
                        nc.scalar.activation(
                            o_sb[:], ps[:], mybir.ActivationFunctionType.Copy,
                            scale=1.0 / SW,
                        )
                        nc.sync.dma_start(orow[:], o_sb[:])
                else:
                    nc.vector.tensor_scalar_mul(o_sb[:], ps[:], 1.0 / SW)
                    nc.scalar.dma_start(orow[:], o_sb[:])

            # ---- prologue: u (3-term) + ob=0 rt0..5, paced per x group ---------
            NRB = R_CORE // NB  # 2 row blocks for u
            ps_l = [psum.tile([UL, NB], F32, tag="ps", name=f"psl{rb}") for rb in range(NRB)]
            ps0 = [psum.tile([128, NB], F32, tag="ps", name=f"ps0_{rt}") for rt in range(6)]
            for g in range(G):
                for rb in range(NRB):
                    rsl = slice(rb * NB, (rb + 1) * NB)
                    nc.tensor.matmul(
                        ps_l[rb][:], ah_sb[:, g], xh_sb[:, g, :, rsl], perf_mode=DR,
                        start=(g == 0), stop=False,
                    )
                for rt in range(6):
                    emit_main_hi(ps0[rt], rt, g, wh0, start=(g == 0))
                for rb in range(NRB):
                    rsl = slice(rb * NB, (rb + 1) * NB)
                    nc.tensor.matmul(
                        ps_l[rb][:], al_sb[:, g], xh_sb[:, g, :, rsl], perf_mode=DR,
                        start=False, stop=False,
                    )
                    nc.tensor.matmul(
                        ps_l[rb][:], ah_sb[:, g], xl_sb[:, g, :, rsl], perf_mode=DR,
                        start=False, stop=(g == G - 1),
                    )
                for rt in range(6):
                    emit_main_lo(ps0[rt], rt, g, wh0, wl0)
            # gate multiply + fp8 hi/lo split into the aug lhsT. The ul-helper
            # muls run on GPSIMD in parallel with the DVE aug8 writes so the
            # serial chain before the first aug matmul is ~4us, not ~7us.
            # DVE: gate muls (psum reads); GPSIMD (no psum access): the
            # sbuf->sbuf fp8-rounding copy that feeds the ul subtraction.
            for rb in range(NRB):
                rsl = slice(rb * NB, (rb + 1) * NB)
                nc.vector.tensor_mul(ug3_sb[32:48, rsl], ps_l[rb][32:48, :], g_sb[32:48, rsl])
                nc.gpsimd.tensor_copy(uh3_sb[32:48, rsl], ug3_sb[32:48, rsl])
                nc.vector.tensor_mul(aug8[0:16, 0, rsl], ps_l[rb][0:16, :], g_sb[0:16, rsl])
                nc.vector.tensor_mul(aug8[0:16, 1, rsl], ps_l[rb][0:16, :], g_sb[0:16, rsl])
            for rb in range(NRB):
                rsl = slice(rb * NB, (rb + 1) * NB)
                nc.vector.tensor_sub(aug8[32:48, 0, rsl], ug3_sb[32:48, rsl], uh3_sb[32:48, rsl])
            for rt in range(6):
                emit_aug(ps0[rt], rt, 0)
                emit_epilogue(ps0[rt], rt, 0)
            for rt in (6, 7):
                ps = psum.tile([128, NB], F32, tag="ps", name=f"ps0b_{rt}")
                for g in range(G):
                    emit_main_hi(ps, rt, g, wh0, start=(g == 0))
                    emit_main_lo(ps, rt, g, wh0, wl0)
                emit_aug(ps, rt, 0)
                emit_epilogue(ps, rt, 0)

            # ---- steady state: ob = 1..7 ---------------------------------------
            for ob in range(1, OB):
                if ob == 1:
                    wh, wl = wh1, wl1  # loaded in the prologue on the ACT ring
                else:
                    wh = wpool.tile([128, G, 2, NB], F8, tag="wh", name=f"wh{ob}")
                    wl = wpool.tile([128, NG2, 2, NB], F8, tag="wl", name=f"wl{ob}")
                    nc.sync.dma_start(wh[:], wh_d.ap()[:, ob])
                    nc.sync.dma_start(wl[:], wl_d.ap()[:, ob])
                for rt in range(RT):
                    ps = psum.tile([128, NB], F32, tag="ps", name=f"ps{ob}_{rt}")
                    for g in range(G):
                        emit_main_hi(ps, rt, g, wh, start=(g == 0))
                        emit_main_lo(ps, rt, g, wh, wl)
                    emit_aug(ps, rt, ob)
                    emit_epilogue(ps, rt, ob)

    nc.compile()
    return nc


_NC_CACHE = None


def _get_nc():
    global _NC_CACHE
    if _NC_CACHE is None:
        _NC_CACHE = _build()
    return _NC_CACHE


def _prep_in_maps(x, W, b, A, B_lora, gates, alpha):
    x = np.asarray(x, dtype=np.float32).reshape(ROWS, D_IN)
    W = np.asarray(W, dtype=np.float32)
    b = np.asarray(b, dtype=np.float32)
    A_last = np.asarray(A, dtype=np.float32)[-1]          # [D_IN, 16]
    B_last = np.asarray(B_lora, dtype=np.float32)[-1]     # [16, D_OUT]
    g_last = np.asarray(gates, dtype=np.float32)[-1].reshape(ROWS)
    alpha_f = float(np.asarray(alpha))

    # ---- shared (replicated) tensors ----
    Wt = W.T * np.float32(SW)                             # [1024, 4096]
    wh8, wl8 = _split2(Wt)
    # pack [1024, 4096] -> [128, G, 2, OB, NB] -> [128, OB, G, 2, NB]
    wh_p = np.ascontiguousarray(
        _pack_k(wh8.reshape(D_IN, OB, NB), (OB, NB)).transpose(0, 3, 1, 2, 4)
    )
    wl_p = np.ascontiguousarray(
        _pack_k(wl8.reshape(D_IN, OB, NB), (OB, NB)).transpose(0, 3, 1, 2, 4)[:, :, list(G2)]
    )

    As = A_last * np.float32(SW)                          # [1024, 16]
    ah8, al8 = _split2(As)
    ah3 = np.concatenate([ah8, ah8, ah8], axis=1)         # [1024, 48]
    al3 = np.concatenate([al8, al8, al8], axis=1)
    ah_p = _pack_k(ah3, (UL,))
    al_p = _pack_k(al3, (UL,))

    # aug rhs pairs (p, i): p0-15: (i0 = Bh paired with uh, i1 = Bl paired
    # with uh); p32-47: (i0 = Bh paired with ul, i1 = 0); p48: bias hi/lo
    # paired with ones; p16-31 unused (zeros both sides).
    Baug = alpha_f * B_last * SW                          # [16, D_OUT]
    Bh8, Bl8 = _split2(Baug)
    bs = b * SW
    bh8, bl8 = _split2(bs)
    rhs8 = np.zeros((KA8, 2, D_OUT), dtype=FP8NP)
    rhs8[0:16, 0] = Bh8
    rhs8[0:16, 1] = Bl8
    rhs8[32:48, 0] = Bh8
    rhs8[48, 0] = bh8
    rhs8[48, 1] = bl8
    ones2 = np.ones((1, 2, R_CORE), dtype=FP8NP)

    in_maps = []
    for c in range(N_CORES):
        rows = slice(c * R_CORE, (c + 1) * R_CORE)
        xs = x[rows]                                      # [R_CORE, D_IN]
        xh8, xl8 = _split2(xs)
        xh_p = _pack_k(np.ascontiguousarray(xh8.T), (R_CORE,))
        xl_p = _pack_k(np.ascontiguousarray(xl8.T), (R_CORE,))
        g_rep = np.ascontiguousarray(
            np.broadcast_to((g_last[rows] / np.float32(SW))[None, :], (UL, R_CORE))
        ).astype(np.float32)
        in_maps.append(
            {
                "xh": xh_p, "xl": xl_p,
                "wh": wh_p, "wl": wl_p,
                "ah": ah_p, "al": al_p,
                "aug_rhs8": rhs8,
                "aug_ones": ones2,
                "g_rep": g_rep,
            }
        )
    return in_maps


def run(inputs: dict, trace: bool = False, trace_cores=None):
    nc = _get_nc()
    in_maps = _prep_in_maps(**inputs)
    res = run_bass_kernel_spmd(
        nc,
        in_maps,
        core_ids=list(range(N_CORES)),
        trace=trace,
        trace_cores=trace_cores,
    )
    out = np.concatenate([np.asarray(r["out"]).astype(np.float32) for r in res.results], axis=0)
    return out.reshape(B, S, D_OUT), res


def kernel(**inputs) -> np.ndarray:
    out, _ = run(inputs, trace=False)
    return out



# revision 20
# speedup vs baseline: 1.1293x; 1.1293x over previous
"""Trainium2 Bass kernel for ClassLinearWithLORA — fp8 DoubleRow, 10-pass.

out = x @ W.T + b + gates[-1] * (alpha * (x @ A[-1]) @ B_lora[-1])

Strategy (per core, data-parallel over 8 cores / 1024 rows each):
  - K=1024 contraction in G=4 groups of 256 K-rows, fp8(e4m3) DoubleRow
    matmuls (256 K-rows per 256-cycle pass).
  - 10 PE passes per [128, 512] psum tile:
      p1-4:  xh.wh per group          (hi term, all K)
      p5-8:  xl.wh per group          (x-residual term, all K — REUSES the
             same wh SBUF tiles, so no extra W traffic)
      p9:    xh.wl on group 3         (w-residual, 256 rows)
      p10:   aug pass: LoRA (uh.Bh, uh.Bl, ul.Bh) + bias (ones.bias_hi/lo)
             on partitions 0-48, plus xh.wl for 158 more K-rows (610..767)
             on partitions 49-127 — w-residual total 414 rows.
    Measured end-to-end rel-err ~1.6e-2 (harness gate 2e-2). W/A/B/b carry
    a global SW=32 pre-scale; epilogue multiplies by 1/SW.
  - u' = SW*g*(x @ A[-1]) via 3-term fp8 DR with A's 16 columns TRIPLICATED
    (ps_l is [48, 512]); DVE gate-multiply writes uh twice + ul once into
    the aug lhsT (partitions 0-48 of augx); partitions 49-127 of augx are
    DMA-loaded xh rows for the fused w-residual slots.
  - DMA batching: one merged W load per output block ([128, 5, 2, 512]:
    4 wh groups + wl g3); output stores batched quad (ob0-3) / triple
    (ob4-6) / single (ob7) per row tile — 24 stores instead of 64. xl rides
    the Pool/SWDGE ring, xh + wc1 the ACT ring, W/smalls the SP ring, so
    the prologue streams three ways in parallel.
  - Tail: ob7 epilogues alternate DVE/ACT engines and both DMA rings.

PE sum = 64*10*256 + 24*256 cyc = 70.8us @ 2.4GHz.
"""

import numpy as np
import ml_dtypes

import concourse.bacc as bacc
import concourse.mybir as mybir
import concourse.tile as tile
from concourse.bass_utils import run_bass_kernel_spmd

F32 = mybir.dt.float32
F16 = mybir.dt.float16
F8 = mybir.dt.float8e4
FP8NP = ml_dtypes.float8_e4m3
DR = mybir.MatmulPerfMode.DoubleRow

N_CORES = 8
B, S, D_IN, D_OUT, R_LORA = 4, 2048, 1024, 4096, 16
ROWS = B * S                  # 8192
R_CORE = ROWS // N_CORES      # 1024 rows per core
G = 4                         # K groups of 256
NB = 512                      # moving free dim
OB = D_OUT // NB              # 8 output blocks
RT = R_CORE // 128            # 8 row tiles per core
UL = 3 * R_LORA               # triplicated u partitions (48)
KA8 = UL + 1                  # aug partitions reserved for LoRA+bias (49)
NXP = 128 - KA8               # extra w-residual partitions in p10 (79)
SEL_LO = 610                  # w-residual rows fused into p10: 610..767
SW = 32.0                     # W-side global scale
E4M3_MAX = 240.0


def _to_fp8(a):
    return np.clip(np.asarray(a, np.float32), -E4M3_MAX, E4M3_MAX).astype(FP8NP)


def _split2(v):
    vh = _to_fp8(v)
    vl = _to_fp8(np.asarray(v, np.float32) - vh.astype(np.float32))
    return vh, vl


def _pack_k(a, tail_shape):
    """[1024K, *tail] -> [128, G, 2, *tail]"""
    return np.ascontiguousarray(
        a.reshape(G, 2, 128, *tail_shape).transpose(2, 0, 1, *range(3, 3 + len(tail_shape)))
    )


def _build(wt_bufs: int = 3, psum_bufs: int = 8):
    nc = bacc.Bacc(None, target_bir_lowering=False)

    pro_d = nc.dram_tensor("pro", [128, 2, 128 + NB], F8, kind="ExternalInput")
    xh_d = nc.dram_tensor("xh", [128, G, 2, R_CORE], F8, kind="ExternalInput")
    xl_d = nc.dram_tensor("xl", [128, G, 2, R_CORE], F8, kind="ExternalInput")
    wc_d = nc.dram_tensor("wc", [128, OB, 5, 2, NB], F8, kind="ExternalInput")
    aa_d = nc.dram_tensor("aa", [128, G, 2, 2, UL], F8, kind="ExternalInput")
    xp_d = nc.dram_tensor("xp10", [NXP, 2, R_CORE], F8, kind="ExternalInput")
    rhsx_d = nc.dram_tensor("rhsx", [128, 2, D_OUT], F8, kind="ExternalInput")
    one_d = nc.dram_tensor("aug_ones", [1, 2, R_CORE], F8, kind="ExternalInput")
    g_d = nc.dram_tensor("g_rep", [UL, R_CORE], F32, kind="ExternalInput")
    out_d = nc.dram_tensor("out", [R_CORE, D_OUT], F16, kind="ExternalOutput")

    with tile.TileContext(nc) as tc:
        with (
            tc.tile_pool(name="resident", bufs=1) as res,
            tc.tile_pool(name="wpool", bufs=wt_bufs) as wpool,
            tc.tile_pool(name="oq", bufs=RT) as oqp,
            tc.tile_pool(name="ot", bufs=RT) as otp,
            tc.tile_pool(name="os", bufs=4) as osp,
            tc.tile_pool(name="psum", bufs=psum_bufs, space="PSUM") as psum,
        ):
            # ---- SP ring: the fused first chunk (xh rt0-rows + wc0 slot0 in
            # ONE transfer so the first matmul waits on a single DMA), then
            # the remaining ob=0 W chunks and aa.
            pro_sb = res.tile([128, 2, 128 + NB], F8)
            nc.sync.dma_start(pro_sb[:], pro_d.ap())
            wc0 = wpool.tile([128, 5, 2, NB], F8, tag="wc", name="wc0")
            aa_sb = res.tile([128, G, 2, 2, UL], F8)
            nc.sync.dma_start(aa_sb[:], aa_d.ap())
            for s in range(1, 5):
                nc.sync.dma_start(wc0[:, s], wc_d.ap()[:, 0, s])

            # aug lhsT: partitions 0-48 DVE-written (zeros/gates/ones),
            # partitions 49-127 DMA-loaded xh rows for fused w-residual.
            augx = res.tile([128, 2, R_CORE], F8)
            nc.vector.memset(augx[0:UL, :, :], 0.0)
            # ones row sits at partition 48: engine writes need partition
            # base 0/32/64/96, so it must arrive via DMA
            nc.sync.dma_start(augx[UL : UL + 1], one_d.ap())
            # scratch for the ul computation (rows 32:48 used)
            uh3_sb = res.tile([UL, R_CORE], F8)
            ug3_sb = res.tile([UL, R_CORE], F32)

            # ---- ACT ring: xh per group (first halved), xp10, smalls, wc1 --
            xh_sb = res.tile([128, G, 2, R_CORE], F8)
            xl_sb = res.tile([128, G, 2, R_CORE], F8)
            hr = R_CORE // 2
            nc.scalar.dma_start(xh_sb[:, 0, :, 0:hr], xh_d.ap()[:, 0, :, 0:hr])
            nc.scalar.dma_start(xh_sb[:, 0, :, hr:R_CORE], xh_d.ap()[:, 0, :, hr:R_CORE])
            for g in range(1, G):
                nc.scalar.dma_start(xh_sb[:, g], xh_d.ap()[:, g])
            nc.scalar.dma_start(augx[KA8:128], xp_d.ap())
            g_sb = res.tile([UL, R_CORE], F32)
            nc.scalar.dma_start(g_sb[:], g_d.ap())
            rhsx_sb = res.tile([128, 2, D_OUT], F8)
            nc.scalar.dma_start(rhsx_sb[:], rhsx_d.ap())

            # ---- Pool/SWDGE ring: xl per group. The tensor_copy below makes
            # the Pool queue wait for the first xh chunk, so xl transfers
            # don't cut ahead of the critical wc0/xh transfers on the shared
            # DMA engines.
            pool_gate = res.tile([1, 4], F8)
            nc.gpsimd.tensor_copy(pool_gate[:], xh_sb[0:1, 0, 0, 0:4])
            for g in range(G):
                nc.gpsimd.dma_start(xl_sb[:, g], xl_d.ap()[:, g])

            # preload the ACT activation table off the critical path
            act_warm = res.tile([1, 16], F32)
            nc.scalar.activation(
                act_warm[:], g_sb[0:1, 0:16], mybir.ActivationFunctionType.Copy,
                scale=1.0,
            )
            # ob=1 weights ride the ACT ring behind xh/xp10
            wc1 = wpool.tile([128, 5, 2, NB], F8, tag="wc", name="wc1")
            nc.scalar.dma_start(wc1[:], wc_d.ap()[:, 1])

            def emit_hi(ps, rt, g, wc, start):
                rs = slice(rt * 128, (rt + 1) * 128)
                if wc is wc0 and g == 0:
                    # ob0/g0 operands live in the fused first chunk
                    lhsT = pro_sb[:, :, 0:128] if rt == 0 else xh_sb[:, 0, :, rs]
                    nc.tensor.matmul(
                        ps[:], lhsT, pro_sb[:, :, 128:], perf_mode=DR,
                        start=start, stop=False,
                    )
                    return
                nc.tensor.matmul(
                    ps[:], xh_sb[:, g, :, rs], wc[:, g], perf_mode=DR,
                    start=start, stop=False,
                )

            def emit_lox(ps, rt, g, wc):
                rs = slice(rt * 128, (rt + 1) * 128)
                rhs = pro_sb[:, :, 128:] if (wc is wc0 and g == 0) else wc[:, g]
                nc.tensor.matmul(
                    ps[:], xl_sb[:, g, :, rs], rhs, perf_mode=DR,
                    start=False, stop=False,
                )

            def emit_p9(ps, rt, wc):
                rs = slice(rt * 128, (rt + 1) * 128)
                nc.tensor.matmul(
                    ps[:], xh_sb[:, 3, :, rs], wc[:, 4], perf_mode=DR,
                    start=False, stop=False,
                )

            def emit_p10(ps, rt, ob):
                nc.tensor.matmul(
                    ps[:],
                    augx[:, :, rt * 128 : (rt + 1) * 128],
                    rhsx_sb[:, :, ob * NB : (ob + 1) * NB],
                    perf_mode=DR,
                    start=False,
                    stop=True,
                )

            oq_tiles = {}
            ot_tiles = {}

            def emit_epilogue(ps, rt, ob):
                orow = out_d.ap()[rt * 128 : (rt + 1) * 128]
                if ob <= 3:
                    if rt not in oq_tiles:
                        oq_tiles[rt] = oqp.tile([128, 4 * NB], F16, tag="oq",
                                                name=f"oq_{rt}")
                    t = oq_tiles[rt]
                    nc.vector.tensor_scalar_mul(t[:, ob * NB : (ob + 1) * NB], ps[:], 1.0 / SW)
                    if ob == 3:
                        dma = nc.scalar.dma_start if rt % 2 == 0 else nc.sync.dma_start
                        dma(orow[:, 0 : 4 * NB], t[:])
                elif ob <= 6:
                    if rt not in ot_tiles:
                        ot_tiles[rt] = otp.tile([128, 3 * NB], F16, tag="ot",
                                                name=f"ot_{rt}")
                    t = ot_tiles[rt]
                    nc.vector.tensor_scalar_mul(t[:, (ob - 4) * NB : (ob - 3) * NB], ps[:], 1.0 / SW)
                    if ob == 6:
                        dma = nc.scalar.dma_start if rt % 2 == 0 else nc.sync.dma_start
                        dma(orow[:, 4 * NB : 7 * NB], t[:])
                elif rt < 6:
                    # tail: alternate compute engine and DMA ring per row tile
                    t = osp.tile([128, NB], F16, tag="os", name=f"os_{rt}")
                    if rt % 2 == 0:
                        nc.vector.tensor_scalar_mul(t[:], ps[:], 1.0 / SW)
                        nc.scalar.dma_start(orow[:, 7 * NB : 8 * NB], t[:])
                    else:
                        nc.scalar.activation(
                            t[:], ps[:], mybir.ActivationFunctionType.Copy,
                            scale=1.0 / SW,
                        )
                        nc.sync.dma_start(orow[:, 7 * NB : 8 * NB], t[:])
                else:
                    # last two tiles: split into column halves on DVE+ACT in
                    # parallel, stores on both rings, to shorten the final
                    # serial drain chain
                    t = osp.tile([128, NB], F16, tag="os", name=f"os_{rt}")
                    h = NB // 2
                    nc.vector.tensor_scalar_mul(t[:, 0:h], ps[:, 0:h], 1.0 / SW)
                    nc.scalar.activation(
                        t[:, h:NB], ps[:, h:NB],
                        mybir.ActivationFunctionType.Copy, scale=1.0 / SW,
                    )
                    nc.scalar.dma_start(orow[:, 7 * NB : 7 * NB + h], t[:, 0:h])
                    nc.sync.dma_start(orow[:, 7 * NB + h : 8 * NB], t[:, h:NB])

            # ---- prologue: u (3-term, early) + ob=0 rt0..3, paced per group.
            # Only 4 main tiles open during the g-loop so 2 psum banks stay
            # free: rt4/rt5 main passes fill the gate-chain window on PE.
            NRB = R_CORE // NB  # 2 row blocks for u
            ps_l = [psum.tile([UL, NB], F32, tag="ps", name=f"psl{rb}") for rb in range(NRB)]
            ps0 = [psum.tile([128, NB], F32, tag="ps", name=f"ps0_{rt}") for rt in range(4)]

            def emit_u(g):
                for rb in range(NRB):
                    rsl = slice(rb * NB, (rb + 1) * NB)
                    nc.tensor.matmul(
                        ps_l[rb][:], aa_sb[:, g, 0], xh_sb[:, g, :, rsl], perf_mode=DR,
                        start=(g == 0), stop=False,
                    )
                    nc.tensor.matmul(
                        ps_l[rb][:], aa_sb[:, g, 1], xh_sb[:, g, :, rsl], perf_mode=DR,
                        start=False, stop=False,
                    )
                    nc.tensor.matmul(
                        ps_l[rb][:], aa_sb[:, g, 0], xl_sb[:, g, :, rsl], perf_mode=DR,
                        start=False, stop=(g == G - 1),
                    )

            for g in range(G):
                if g == 0:
                    for rt in range(4):
                        emit_hi(ps0[rt], rt, g, wc0, start=True)
                    emit_u(g)
                else:
                    emit_u(g)
                    for rt in range(4):
                        emit_hi(ps0[rt], rt, g, wc0, start=False)
                for rt in range(4):
                    emit_lox(ps0[rt], rt, g, wc0)

            # gate multiply + fp8 hi/lo split into the aug lhsT. DVE does the
            # psum reads (gate muls); GPSIMD handles the sbuf-only fp8 lane
            # duplicate, rounding copy, and residual subtract, per row block
            # (rb0 first — it unblocks p10 for rt0-3).
            for rb in range(NRB):
                rsl = slice(rb * NB, (rb + 1) * NB)
                nc.vector.tensor_mul(ug3_sb[32:48, rsl], ps_l[rb][32:48, :], g_sb[32:48, rsl])
                nc.gpsimd.tensor_copy(uh3_sb[32:48, rsl], ug3_sb[32:48, rsl])
                nc.vector.tensor_mul(augx[0:16, 0, rsl], ps_l[rb][0:16, :], g_sb[0:16, rsl])
                nc.vector.tensor_mul(augx[0:16, 1, rsl], ps_l[rb][0:16, :], g_sb[0:16, rsl])
                nc.vector.tensor_sub(augx[32:48, 0, rsl], ug3_sb[32:48, rsl], uh3_sb[32:48, rsl])

            # PE filler while the gate chain runs
            for rt in range(4):
                emit_p9(ps0[rt], rt, wc0)
            ps45 = []
            for rt in (4, 5):
                ps = psum.tile([128, NB], F32, tag="ps", name=f"ps0m_{rt}")
                for g in range(G):
                    emit_hi(ps, rt, g, wc0, start=(g == 0))
                    emit_lox(ps, rt, g, wc0)
                emit_p9(ps, rt, wc0)
                ps45.append(ps)
            for rt in range(4):
                emit_p10(ps0[rt], rt, 0)
                emit_epilogue(ps0[rt], rt, 0)
            for rt in (4, 5):
                emit_p10(ps45[rt - 4], rt, 0)
                emit_epilogue(ps45[rt - 4], rt, 0)
            for rt in (6, 7):
                ps = psum.tile([128, NB], F32, tag="ps", name=f"ps0b_{rt}")
                for g in range(G):
                    emit_hi(ps, rt, g, wc0, start=(g == 0))
                    emit_lox(ps, rt, g, wc0)
                emit_p9(ps, rt, wc0)
                emit_p10(ps, rt, 0)
                emit_epilogue(ps, rt, 0)

            # ---- steady state: ob = 1..7 ---------------------------------------
            for ob in range(1, OB):
                if ob == 1:
                    wc = wc1  # loaded in the prologue on the ACT ring
                else:
                    wc = wpool.tile([128, 5, 2, NB], F8, tag="wc", name=f"wc{ob}")
                    nc.sync.dma_start(wc[:], wc_d.ap()[:, ob])
                for rt in range(RT):
                    ps = psum.tile([128, NB], F32, tag="ps", name=f"ps{ob}_{rt}")
                    for g in range(G):
                        emit_hi(ps, rt, g, wc, start=(g == 0))
                        emit_lox(ps, rt, g, wc)
                    emit_p9(ps, rt, wc)
                    emit_p10(ps, rt, ob)
                    emit_epilogue(ps, rt, ob)

    nc.compile()
    return nc


_NC_CACHE = None


def _get_nc():
    global _NC_CACHE
    if _NC_CACHE is None:
        _NC_CACHE = _build()
    return _NC_CACHE


def _prep_in_maps(x, W, b, A, B_lora, gates, alpha):
    x = np.asarray(x, dtype=np.float32).reshape(ROWS, D_IN)
    W = np.asarray(W, dtype=np.float32)
    b = np.asarray(b, dtype=np.float32)
    A_last = np.asarray(A, dtype=np.float32)[-1]          # [D_IN, 16]
    B_last = np.asarray(B_lora, dtype=np.float32)[-1]     # [16, D_OUT]
    g_last = np.asarray(gates, dtype=np.float32)[-1].reshape(ROWS)
    alpha_f = float(np.asarray(alpha))

    # ---- shared (replicated) tensors ----
    Wt = W.T * np.float32(SW)                             # [1024, 4096]
    wh8, wl8 = _split2(Wt)
    # wc: [128, OB, 5, 2, NB] — slots 0-3 wh per group, slot 4 wl g3
    wh_p = _pack_k(wh8.reshape(D_IN, OB, NB), (OB, NB))   # [128, G, 2, OB, NB]
    wc = np.empty((128, OB, 5, 2, NB), dtype=FP8NP)
    wc[:, :, 0:4] = wh_p.transpose(0, 3, 1, 2, 4)
    wl_g3 = wl8[3 * 256 : 4 * 256].reshape(2, 128, OB, NB)
    wc[:, :, 4] = wl_g3.transpose(1, 2, 0, 3)             # [128, OB, 2, NB]

    As = A_last * np.float32(SW)                          # [1024, 16]
    ah8, al8 = _split2(As)
    ah3 = np.concatenate([ah8, ah8, ah8], axis=1)         # [1024, 48]
    al3 = np.concatenate([al8, al8, al8], axis=1)
    ah_p = _pack_k(ah3, (UL,))
    al_p = _pack_k(al3, (UL,))
    aa = np.ascontiguousarray(np.stack([ah_p, al_p], axis=2))  # [128, G, 2, 2, UL]

    # p10 rhs: rows 0-48 LoRA/bias aug, rows 49-127 wl for rows 610..767
    Baug = alpha_f * B_last * SW                          # [16, D_OUT]
    Bh8, Bl8 = _split2(Baug)
    bs = b * SW
    bh8, bl8 = _split2(bs)
    rhsx = np.zeros((128, 2, D_OUT), dtype=FP8NP)
    rhsx[0:16, 0] = Bh8
    rhsx[0:16, 1] = Bl8
    rhsx[32:48, 0] = Bh8
    rhsx[48, 0] = bh8
    rhsx[48, 1] = bl8
    rhsx[KA8:128] = wl8[SEL_LO : SEL_LO + 2 * NXP].reshape(NXP, 2, D_OUT)

    in_maps = []
    for c in range(N_CORES):
        rows = slice(c * R_CORE, (c + 1) * R_CORE)
        xs = x[rows]                                      # [R_CORE, D_IN]
        xh8, xl8 = _split2(xs)
        xh_t = np.ascontiguousarray(xh8.T)                # [1024, R_CORE]
        xh_p = _pack_k(xh_t, (R_CORE,))
        xl_p = _pack_k(np.ascontiguousarray(xl8.T), (R_CORE,))
        xp10 = np.ascontiguousarray(
            xh_t[SEL_LO : SEL_LO + 2 * NXP].reshape(NXP, 2, R_CORE)
        )
        # fused first chunk: [xh g0 rows 0:128 | wc ob0 slot0]
        pro = np.concatenate([xh_p[:, 0, :, 0:128], wc[:, 0, 0]], axis=-1)
        pro = np.ascontiguousarray(pro)
        g_rep = np.ascontiguousarray(
            np.broadcast_to((g_last[rows] / np.float32(SW))[None, :], (UL, R_CORE))
        ).astype(np.float32)
        in_maps.append(
            {
                "pro": pro,
                "xh": xh_p, "xl": xl_p,
                "wc": wc, "aa": aa,
                "aug_ones": np.ones((1, 2, R_CORE), dtype=FP8NP),
                "xp10": xp10, "rhsx": rhsx,
                "g_rep": g_rep,
            }
        )
    return in_maps


def run(inputs: dict, trace: bool = False, trace_cores=None):
    nc = _get_nc()
    in_maps = _prep_in_maps(**inputs)
    res = run_bass_kernel_spmd(
        nc,
        in_maps,
        core_ids=list(range(N_CORES)),
        trace=trace,
        trace_cores=trace_cores,
    )
    out = np.concatenate([np.asarray(r["out"]).astype(np.float32) for r in res.results], axis=0)
    return out.reshape(B, S, D_OUT), res


def kernel(**inputs) -> np.ndarray:
    out, _ = run(inputs, trace=False)
    return out
